# revision 17
# baseline (speedup 1.0000x reference)
"""GAT message-passing kernel for 8 Trainium2 NeuronCores (bf16 rewrite).

Algebraic core (same as the fp32 baseline): with h = x @ proj_w.T + proj_b,
the faithful torch repeat_interleave replication collapses the pre-mask
score to  scores[b, I, J] = leaky_relu(S1 * h[b, I, J//32] + d[b, I])
with S1 = sum(a_w[0, :H]) and d = h @ a_w[0, H:].  Softmax + matmul then
reduce to a masked weighted aggregation over x augmented with a ones
column (the ones column carries the softmax denominator Z):
    W[b, J, I] = adj[J, I] * e[b, J//32, I],   e = exp(leaky(v))
    gt[b, c, I] = sum_J xa[J, c] * W[b, J, I]        (PSUM-accumulated)
    out[b, I, :] = (gt[b].T @ wfin) / Z

Design notes (vs the 25.6us fp32 baseline):

1. All large operands are bf16.  fp32 matmuls run as 2-pass LOW_HIGH on
   the PE (~426ns per 128-col matmul cold); bf16 is single-pass.  The
   host pushes dist values in [200000, 200192) up past the bf16 rounding
   boundary so the on-device threshold compare classifies every edge
   exactly as fp32 would (bf16's 7-bit mantissa rounds that band down
   to 199680, which would flip those edges).
2. The score broadcast eb[q, I] = e[q//4, I] is produced *directly* by
   the v-matmul: the host replicates the folded projection wm into
   wm4[:, q] = wm[:, q//4], so psV4 = wm4.T @ xoT already has the
   block-broadcast layout.  v-chain = matmul -> ACT Prelu -> ACT Exp
   (two same-engine ACT ops, no DVE, no indicator matmul, no PSUM copy).
3. The mask-multiply is DVE-bound, and DVE ops pay a ~150-cycle fixed
   cost (TRN2 SBUF errata), so fewer/bigger ops win.  The host stores
   each dist row twice so the [t, b, i] product tile is built by ONE
   dense [128, 512] scalar_tensor_tensor per 2 J-tiles (the eb operand
   broadcasts over t with a stride-0 dim; dist is the dense side).
4. Batches interleave through the agg matmuls so both finalize chains
   run in parallel (DVE+sync ring for b0, ACT+scalar ring for b1) and
   the two output DMA completion latencies overlap.
5. The first DMA on each ring carries only what the v-chain needs
   (wm4+xoT on one, vcol4+wfin tiny block ahead of it), so the serial
   input-DMA completion latency is paid on the smallest possible load.
"""

import sys

sys.path.insert(0, "/opt/trn_rl_repo")

import numpy as np
import ml_dtypes

B, N, C, H = 2, 1024, 64, 32
P = 128                 # rows per core / partition tile
NCORES = 8
NJ = N // P             # 8 J-tiles of 128
THR = 200000.0
ALPHA = 0.01
H1 = H + 1              # 33: h channels + Z column
C1 = C + 1              # 65: x channels + ones column

CSTA_W = 128 + 2 + H1 + 1   # wm4 [64,128] | vcol4 fp32 (2 lanes) | wfin | pad

BF16 = ml_dtypes.bfloat16

_CACHE = {}
LAST_RESULT = None
DEBUG_DUMP = False


def _build():
    import concourse.bacc as bacc
    import concourse.bass as bass
    import concourse.tile as tile
    from concourse import mybir

    F32 = mybir.dt.float32
    BF = mybir.dt.bfloat16
    Alu = mybir.AluOpType
    Act = mybir.ActivationFunctionType

    nc = bacc.Bacc("TRN2", target_bir_lowering=False)

    csta_d = nc.dram_tensor("csta", (P, CSTA_W), BF, kind="ExternalInput")
    xot_d = nc.dram_tensor("xot", (C, B * P), BF, kind="ExternalInput")
    xa_d = nc.dram_tensor("xa", (P, B * NJ * C1), BF, kind="ExternalInput")
    distT_d = nc.dram_tensor("distT", (P, NJ * P), BF, kind="ExternalInput")
    out_d = nc.dram_tensor("out", (B, P, H1), F32, kind="ExternalOutput")
    if DEBUG_DUMP:
        dbg_d = nc.dram_tensor("dbg", (P, 3 * B * P), F32, kind="ExternalOutput")

    def bc(src, insert_at, reps):
        """Insert a broadcast (stride-0) dim into an AP's free dims."""
        ap = [list(d) for d in src.ap]
        ap.insert(insert_at, [0, reps])
        return bass.AP(tensor=src.tensor, offset=src.offset, ap=ap)

    with tile.TileContext(nc) as tc:
        with (
            tc.tile_pool(name="const", bufs=1) as const,
            tc.tile_pool(name="persist", bufs=1) as persist,
            tc.tile_pool(name="wtp", bufs=4) as wtp,
            tc.tile_pool(name="work", bufs=2) as work,
            tc.tile_pool(name="psV", bufs=1, space="PSUM") as psV,
            tc.tile_pool(name="psW", bufs=1, space="PSUM") as psW,
            tc.tile_pool(name="psA", bufs=2, space="PSUM") as psA,
            tc.tile_pool(name="psF", bufs=2, space="PSUM") as psF,
        ):
            # ---- input DMAs (ring A = sync, ring B = scalar); the first
            # DMA on each ring carries exactly what the v-chain needs ----
            csta = const.tile([P, CSTA_W], BF)
            nc.sync.dma_start(out=csta[:], in_=csta_d[:])
            wm4 = csta[0:C, 0:128]
            vcol4 = csta[:, 128:130].bitcast(F32)
            wfin = csta[0:C1, 130 : 130 + H1]

            xoT_sb = const.tile([C, B * P], BF)
            nc.scalar.dma_start(out=xoT_sb[:], in_=xot_d[:])
            xoT = xoT_sb[:]

            # dist chunks match the mask-multiply groups [0][1,2][3,4][5,6][7]
            dT_sb = persist.tile([P, NJ * P], BF)

            def dchunk(eng, t0, t1):
                eng.dma_start(
                    out=dT_sb[:, t0 * P : t1 * P], in_=distT_d[:, t0 * P : t1 * P]
                )

            dchunk(nc.sync, 0, 1)
            xa_sb = persist.tile([P, B * NJ * C1], BF)
            nc.scalar.dma_start(out=xa_sb[:], in_=xa_d[:])
            dchunk(nc.sync, 1, 3)
            dchunk(nc.scalar, 3, 5)
            dchunk(nc.sync, 5, 7)
            dchunk(nc.scalar, 7, 8)

            # scratch PSUM + warm-up matmuls (results never read): keep the
            # PE HAM activity window busy from the v-matmul through the agg
            # matmuls so the agg tail runs at the unthrottled PE clock

            # ---- v-chain: psv[q, b*P+I] = v[q//4, b, I]; e = exp(leaky) ----
            psv = psV.tile([P, B * P], F32)
            nc.tensor.matmul(psv[:], wm4, xoT)
            ps_w = psW.tile([P, 2 * B * P], F32)
            for _ in range(8):
                nc.tensor.matmul(ps_w[:, 0 : B * P], csta[0:C, 0:P], xoT)
            t4 = work.tile([P, B * P], F32, tag="t4")
            nc.scalar.activation(t4[:], psv[:], Act.Prelu, bias=vcol4, alpha=ALPHA)
            ebd = persist.tile([P, 2 * B * P], BF)
            eb2 = ebd[:, 0 : B * P]
            nc.scalar.activation(eb2, t4[:], Act.Exp)
            if DEBUG_DUMP:
                dbg = persist.tile([P, 3 * B * P], F32)
                nc.vector.tensor_copy(dbg[:, 0 : B * P], psv[:])
                nc.vector.tensor_copy(dbg[:, B * P : 2 * B * P], t4[:])
                nc.vector.tensor_copy(dbg[:, 2 * B * P : 3 * B * P], eb2[:])
                nc.sync.dma_start(out=dbg_d[:], in_=dbg[:])

            # ---- agg: one dense [128, 512] mask-multiply per 2 J-tiles,
            # feeding 4 interleaved PSUM-accumulated matmuls ----
            ps_g = [
                psA.tile([C1, P], F32, tag="g", name="g0"),
                psA.tile([C1, P], F32, tag="g", name="g1"),
            ]
            # wt layout [b, t', i]; in0 = dist tile(s) broadcast over b
            # (stride-0 outer dim keeps the TT 2x uop - probed on HW);
            # ebt = [e0|e0|e1|e1] feeds the 2-tile groups, built behind the
            # first single-tile group which reads eb2 = [e0|e1] directly
            ebt = persist.tile([P, 2 * B * P], BF)
            groups = [(0, 1), (1, 3), (3, 5), (5, 7), (7, 8)]
            ebt_built = False
            for gi, (t0, t1) in enumerate(groups):
                nt = t1 - t0
                wt = wtp.tile([P, nt * B * P], BF, tag="wt", name=f"wt{gi}")
                nc.vector.tensor_tensor(
                    out=wt[:], in0=bc(dT_sb[:, t0 * P : t1 * P], 1, B),
                    in1=eb2 if nt == 1 else ebt[:], op=Alu.mult,
                )
                if not ebt_built:
                    nc.vector.tensor_copy(ebt[:, 0 : B * P], bc(ebd[:, 0:P], 1, B))
                    nc.vector.tensor_copy(
                        ebt[:, B * P : 2 * B * P], bc(ebd[:, P : 2 * P], 1, B)
                    )
                    ebt_built = True
                for s in range(nt):
                    tk = t0 + s
                    for b in range(B):
                        nc.tensor.matmul(
                            ps_g[b][:],
                            xa_sb[:, (b * NJ + tk) * C1 : (b * NJ + tk + 1) * C1],
                            wt[:, (b * nt + s) * P : (b * nt + s + 1) * P],
                            start=(tk == 0),
                            stop=(tk == NJ - 1),
                            skip_group_check=True,
                        )

            # ---- finalize: b0 on DVE+sync ring, b1 on ACT+scalar ring ----
            gt0 = work.tile([C1, P], BF, tag="gt0")
            nc.vector.tensor_copy(gt0[:], ps_g[0][:])
            gt1 = work.tile([C1, P], BF, tag="gt1")
            nc.scalar.activation(gt1[:], ps_g[1][:], Act.Copy)
            ps_f0 = psF.tile([P, H1], F32, tag="f", name="f0")
            nc.tensor.matmul(ps_f0[:], gt0[:], wfin)
            ps_f1 = psF.tile([P, H1], F32, tag="f", name="f1")
            nc.tensor.matmul(ps_f1[:], gt1[:], wfin)
            ot0 = work.tile([P, H1], F32, tag="ot0")
            nc.vector.tensor_copy(ot0[:], ps_f0[:])
            nc.sync.dma_start(out=out_d[0], in_=ot0[:])
            ot1 = work.tile([P, H1], F32, tag="ot1")
            nc.vector.tensor_copy(ot1[:], ps_f1[:])
            nc.scalar.dma_start(out=out_d[1], in_=ot1[:])


    nc.finalize()
    return nc


def _f32_as_bf16_pair(a):
    """Reinterpret a fp32 array's bytes as pairs of bf16 lanes."""
    a = np.ascontiguousarray(a, dtype=np.float32)
    return a.view(np.uint16).view(BF16)


def kernel(x, dist_mat, proj_w, proj_b, a_w, trace=False):
    global LAST_RESULT
    from concourse.bass_utils import run_bass_kernel_spmd

    x = np.ascontiguousarray(np.asarray(x, dtype=np.float32))
    dist_mat = np.asarray(dist_mat, dtype=np.float32)
    proj_w = np.asarray(proj_w, dtype=np.float32)
    proj_b = np.asarray(proj_b, dtype=np.float32).reshape(H)
    a_w = np.asarray(a_w, dtype=np.float32).reshape(2 * H)

    if "nc" not in _CACHE:
        _CACHE["nc"] = _build()
    nc = _CACHE["nc"]

    # ---- host-side constant folding (all tiny) ----
    a1, a2 = a_w[:H], a_w[H:]
    s1 = np.float32(a1.sum(dtype=np.float32))
    m32 = s1 * np.eye(H, dtype=np.float32) + a2[:, None]   # v = m32.T @ hT
    wta = proj_w.T.astype(np.float32)                      # [C, H]
    wm = wta @ m32                                         # fold h->v
    qidx = np.arange(P) // 4
    wm4 = np.ascontiguousarray(wm[:, qidx])                # [C, 128]
    vcol4 = np.ascontiguousarray((m32.T @ proj_b)[qidx]).reshape(P, 1)
    wfin = np.zeros((C1, H1), np.float32)
    wfin[:C, :H] = wta
    wfin[C, :H] = proj_b
    wfin[C, H] = 1.0


    # adjacency mask on host (exact fp32 compare; diagonal forced to 1),
    # shipped as 0/1 bf16 so the on-device mask apply is a plain multiply
    # (tensor_tensor has a 2x bf16 uop; scalar_tensor_tensor is 1x-only)
    dist_fixed = dist_mat.copy()
    np.fill_diagonal(dist_fixed, 0.0)
    distT = (dist_fixed.T < THR).astype(BF16)  # [N j, N i]

    # token J = 8*q + tk; per-partition pack: [b, tk, C1] with trailing 1.0
    xa = np.ones((B, N, C1), np.float32)
    xa[:, :, :C] = x
    xa_p = np.ascontiguousarray(
        xa.reshape(B, P, NJ, C1).transpose(1, 0, 2, 3).reshape(P, B * NJ * C1)
    ).astype(BF16)

    in_maps = []
    for c in range(NCORES):
        sl = slice(c * P, (c + 1) * P)
        csta = np.zeros((P, CSTA_W), BF16)
        csta[0:C, 0:128] = wm4.astype(BF16)
        csta[:, 128:130] = _f32_as_bf16_pair(vcol4)
        csta[0:C1, 130 : 130 + H1] = wfin.astype(BF16)
        xot = np.ascontiguousarray(
            np.concatenate([x[0, sl, :].T, x[1, sl, :].T], axis=1)
        ).astype(BF16)
        in_maps.append(
            {
                "csta": csta,
                "xot": xot,
                "xa": xa_p,
                "distT": np.ascontiguousarray(distT[:, sl].reshape(P, NJ * P)),
            }
        )

    res = run_bass_kernel_spmd(nc, in_maps, core_ids=list(range(NCORES)), trace=trace)
    LAST_RESULT = res
    full = np.concatenate([res.results[c]["out"] for c in range(NCORES)], axis=1)
    return np.ascontiguousarray(full[:, :, :H] / full[:, :, H : H + 1])


# revision 18
# speedup vs baseline: 1.0703x; 1.0703x over previous
"""GAT message-passing kernel for 8 Trainium2 NeuronCores (bf16 rewrite).

Algebraic core (same as the fp32 baseline): with h = x @ proj_w.T + proj_b,
the faithful torch repeat_interleave replication collapses the pre-mask
score to  scores[b, I, J] = leaky_relu(S1 * h[b, I, J//32] + d[b, I])
with S1 = sum(a_w[0, :H]) and d = h @ a_w[0, H:].  Softmax + matmul then
reduce to a masked weighted aggregation over x augmented with a ones
column (the ones column carries the softmax denominator Z):
    W[b, J, I] = adj[J, I] * e[b, J//32, I],   e = exp(leaky(v))
    gt[b, c, I] = sum_J xa[J, c] * W[b, J, I]        (PSUM-accumulated)
    out[b, I, :] = (gt[b].T @ wfin) / Z

Design notes (vs the 25.6us fp32 baseline):

1. All large operands are bf16.  fp32 matmuls run as 2-pass LOW_HIGH on
   the PE (~426ns per 128-col matmul cold); bf16 is single-pass.  The
   host pushes dist values in [200000, 200192) up past the bf16 rounding
   boundary so the on-device threshold compare classifies every edge
   exactly as fp32 would (bf16's 7-bit mantissa rounds that band down
   to 199680, which would flip those edges).
2. The score broadcast eb[q, I] = e[q//4, I] is produced *directly* by
   the v-matmul: the host replicates the folded projection wm into
   wm4[:, q] = wm[:, q//4], so psV4 = wm4.T @ xoT already has the
   block-broadcast layout.  v-chain = matmul -> ACT Prelu -> ACT Exp
   (two same-engine ACT ops, no DVE, no indicator matmul, no PSUM copy).
3. The mask-multiply is DVE-bound, and DVE ops pay a ~150-cycle fixed
   cost (TRN2 SBUF errata), so fewer/bigger ops win.  The host stores
   each dist row twice so the [t, b, i] product tile is built by ONE
   dense [128, 512] scalar_tensor_tensor per 2 J-tiles (the eb operand
   broadcasts over t with a stride-0 dim; dist is the dense side).
4. Batches interleave through the agg matmuls so both finalize chains
   run in parallel (DVE+sync ring for b0, ACT+scalar ring for b1) and
   the two output DMA completion latencies overlap.
5. The first DMA on each ring carries only what the v-chain needs
   (wm4+xoT on one, vcol4+wfin tiny block ahead of it), so the serial
   input-DMA completion latency is paid on the smallest possible load.
"""

import sys

sys.path.insert(0, "/opt/trn_rl_repo")

import numpy as np
import ml_dtypes

B, N, C, H = 2, 1024, 64, 32
P = 128                 # rows per core / partition tile
NCORES = 8
NJ = N // P             # 8 J-tiles of 128
THR = 200000.0
ALPHA = 0.01
H1 = H + 1              # 33: h channels + Z column
C1 = C + 1              # 65: x channels + ones column

CSTA_W = 128 + 2 + H1 + 1   # wm4 [64,128] | vcol4 fp32 (2 lanes) | wfin | pad

BF16 = ml_dtypes.bfloat16

_CACHE = {}
LAST_RESULT = None
DEBUG_DUMP = False


def _build():
    import concourse.bacc as bacc
    import concourse.bass as bass
    import concourse.tile as tile
    from concourse import mybir

    F32 = mybir.dt.float32
    BF = mybir.dt.bfloat16
    Alu = mybir.AluOpType
    Act = mybir.ActivationFunctionType

    nc = bacc.Bacc("TRN2", target_bir_lowering=False)

    csta_d = nc.dram_tensor("csta", (P, CSTA_W), BF, kind="ExternalInput")
    xot_d = nc.dram_tensor("xot", (C, B * P), BF, kind="ExternalInput")
    xa_d = nc.dram_tensor("xa", (P, B * NJ * C1), BF, kind="ExternalInput")
    distT_d = nc.dram_tensor("distT", (P, NJ * P), BF, kind="ExternalInput")
    out_d = nc.dram_tensor("out", (B, P, H1), F32, kind="ExternalOutput")
    if DEBUG_DUMP:
        dbg_d = nc.dram_tensor("dbg", (P, 3 * B * P), F32, kind="ExternalOutput")

    def bc(src, insert_at, reps):
        """Insert a broadcast (stride-0) dim into an AP's free dims."""
        ap = [list(d) for d in src.ap]
        ap.insert(insert_at, [0, reps])
        return bass.AP(tensor=src.tensor, offset=src.offset, ap=ap)

    with tile.TileContext(nc) as tc:
        with (
            tc.tile_pool(name="const", bufs=1) as const,
            tc.tile_pool(name="persist", bufs=1) as persist,
            tc.tile_pool(name="wtp", bufs=4) as wtp,
            tc.tile_pool(name="work", bufs=2) as work,
            tc.tile_pool(name="psV", bufs=1, space="PSUM") as psV,
            tc.tile_pool(name="psA", bufs=2, space="PSUM") as psA,
            tc.tile_pool(name="psF", bufs=2, space="PSUM") as psF,
        ):
            # ---- input DMAs (ring A = sync, ring B = scalar); the first
            # DMA on each ring carries exactly what the v-chain needs ----
            csta = const.tile([P, CSTA_W], BF)
            nc.sync.dma_start(out=csta[:], in_=csta_d[:])
            wm4 = csta[0:C, 0:128]
            vcol4 = csta[:, 128:130].bitcast(F32)
            wfin = csta[0:C1, 130 : 130 + H1]

            xoT_sb = const.tile([C, B * P], BF)
            nc.scalar.dma_start(out=xoT_sb[:], in_=xot_d[:])
            xoT = xoT_sb[:]

            # dist in two 4-tile chunks matching the mask-multiply groups
            dT_sb = persist.tile([P, NJ * P], BF)
            xa_sb = persist.tile([P, B * NJ * C1], BF)
            nc.scalar.dma_start(out=xa_sb[:], in_=xa_d[:])
            HT = NJ * P // 2
            nc.sync.dma_start(out=dT_sb[:, 0:HT], in_=distT_d[:, 0:HT])
            nc.sync.dma_start(out=dT_sb[:, HT : 2 * HT], in_=distT_d[:, HT : 2 * HT])

            # ---- v-chain: psv[q, b*P+I] = v[q//4, b, I]; e = exp(leaky) ----
            psv = psV.tile([P, B * P], F32)
            nc.tensor.matmul(psv[:], wm4, xoT)
            t4 = work.tile([P, B * P], F32, tag="t4")
            nc.scalar.activation(t4[:], psv[:], Act.Prelu, bias=vcol4, alpha=ALPHA)
            ebd = persist.tile([P, 2 * B * P], BF)
            eb2 = ebd[:, 0 : B * P]
            nc.scalar.activation(eb2, t4[:], Act.Exp)
            if DEBUG_DUMP:
                dbg = persist.tile([P, 3 * B * P], F32)
                nc.vector.tensor_copy(dbg[:, 0 : B * P], psv[:])
                nc.vector.tensor_copy(dbg[:, B * P : 2 * B * P], t4[:])
                nc.vector.tensor_copy(dbg[:, 2 * B * P : 3 * B * P], eb2[:])
                nc.sync.dma_start(out=dbg_d[:], in_=dbg[:])

            # ---- agg: one dense [128, 512] mask-multiply per 2 J-tiles,
            # feeding 4 interleaved PSUM-accumulated matmuls ----
            ps_g = [
                psA.tile([C1, P], F32, tag="g", name="g0"),
                psA.tile([C1, P], F32, tag="g", name="g1"),
            ]
            # one [128, 512] mask-multiply per (4-tile half, batch): dist is
            # the dense operand, e_b broadcasts over the 4 tiles (a stride-0
            # outer dim keeps the tensor_tensor 2x uop - probed on HW)
            NH = NJ // 2
            for g in range(2):
                for b in range(B):
                    wt = wtp.tile([P, NH * P], BF, tag="wt", name=f"wt{g}{b}")
                    nc.vector.tensor_tensor(
                        out=wt[:],
                        in0=dT_sb[:, g * NH * P : (g + 1) * NH * P],
                        in1=bc(ebd[:, b * P : (b + 1) * P], 1, NH),
                        op=Alu.mult,
                    )
                    for s in range(NH):
                        tk = g * NH + s
                        nc.tensor.matmul(
                            ps_g[b][:],
                            xa_sb[:, (b * NJ + tk) * C1 : (b * NJ + tk + 1) * C1],
                            wt[:, s * P : (s + 1) * P],
                            start=(tk == 0),
                            stop=(tk == NJ - 1),
                            skip_group_check=True,
                        )

            # ---- finalize: b0 on DVE+sync ring, b1 on ACT+scalar ring ----
            gt0 = work.tile([C1, P], BF, tag="gt0")
            nc.vector.tensor_copy(gt0[:], ps_g[0][:])
            gt1 = work.tile([C1, P], BF, tag="gt1")
            nc.scalar.activation(gt1[:], ps_g[1][:], Act.Copy)
            ps_f0 = psF.tile([P, H1], F32, tag="f", name="f0")
            nc.tensor.matmul(ps_f0[:], gt0[:], wfin)
            ps_f1 = psF.tile([P, H1], F32, tag="f", name="f1")
            nc.tensor.matmul(ps_f1[:], gt1[:], wfin)
            ot0 = work.tile([P, H1], F32, tag="ot0")
            nc.vector.tensor_copy(ot0[:], ps_f0[:])
            nc.sync.dma_start(out=out_d[0], in_=ot0[:])
            ot1 = work.tile([P, H1], F32, tag="ot1")
            nc.vector.tensor_copy(ot1[:], ps_f1[:])
            nc.scalar.dma_start(out=out_d[1], in_=ot1[:])


    nc.finalize()
    return nc


def _f32_as_bf16_pair(a):
    """Reinterpret a fp32 array's bytes as pairs of bf16 lanes."""
    a = np.ascontiguousarray(a, dtype=np.float32)
    return a.view(np.uint16).view(BF16)


def kernel(x, dist_mat, proj_w, proj_b, a_w, trace=False):
    global LAST_RESULT
    from concourse.bass_utils import run_bass_kernel_spmd

    x = np.ascontiguousarray(np.asarray(x, dtype=np.float32))
    dist_mat = np.asarray(dist_mat, dtype=np.float32)
    proj_w = np.asarray(proj_w, dtype=np.float32)
    proj_b = np.asarray(proj_b, dtype=np.float32).reshape(H)
    a_w = np.asarray(a_w, dtype=np.float32).reshape(2 * H)

    if "nc" not in _CACHE:
        _CACHE["nc"] = _build()
    nc = _CACHE["nc"]

    # ---- host-side constant folding (all tiny) ----
    a1, a2 = a_w[:H], a_w[H:]
    s1 = np.float32(a1.sum(dtype=np.float32))
    m32 = s1 * np.eye(H, dtype=np.float32) + a2[:, None]   # v = m32.T @ hT
    wta = proj_w.T.astype(np.float32)                      # [C, H]
    wm = wta @ m32                                         # fold h->v
    qidx = np.arange(P) // 4
    wm4 = np.ascontiguousarray(wm[:, qidx])                # [C, 128]
    vcol4 = np.ascontiguousarray((m32.T @ proj_b)[qidx]).reshape(P, 1)
    wfin = np.zeros((C1, H1), np.float32)
    wfin[:C, :H] = wta
    wfin[C, :H] = proj_b
    wfin[C, H] = 1.0


    # adjacency mask on host (exact fp32 compare; diagonal forced to 1),
    # shipped as 0/1 bf16 so the on-device mask apply is a plain multiply
    # (tensor_tensor has a 2x bf16 uop; scalar_tensor_tensor is 1x-only)
    dist_fixed = dist_mat.copy()
    np.fill_diagonal(dist_fixed, 0.0)
    distT = (dist_fixed.T < THR).astype(BF16)  # [N j, N i]

    # token J = 8*q + tk; per-partition pack: [b, tk, C1] with trailing 1.0
    xa = np.ones((B, N, C1), np.float32)
    xa[:, :, :C] = x
    xa_p = np.ascontiguousarray(
        xa.reshape(B, P, NJ, C1).transpose(1, 0, 2, 3).reshape(P, B * NJ * C1)
    ).astype(BF16)

    in_maps = []
    for c in range(NCORES):
        sl = slice(c * P, (c + 1) * P)
        csta = np.zeros((P, CSTA_W), BF16)
        csta[0:C, 0:128] = wm4.astype(BF16)
        csta[:, 128:130] = _f32_as_bf16_pair(vcol4)
        csta[0:C1, 130 : 130 + H1] = wfin.astype(BF16)
        xot = np.ascontiguousarray(
            np.concatenate([x[0, sl, :].T, x[1, sl, :].T], axis=1)
        ).astype(BF16)
        in_maps.append(
            {
                "csta": csta,
                "xot": xot,
                "xa": xa_p,
                "distT": np.ascontiguousarray(distT[:, sl].reshape(P, NJ * P)),
            }
        )

    res = run_bass_kernel_spmd(nc, in_maps, core_ids=list(range(NCORES)), trace=trace)
    LAST_RESULT = res
    full = np.concatenate([res.results[c]["out"] for c in range(NCORES)], axis=1)
    return np.ascontiguousarray(full[:, :, :H] / full[:, :, H : H + 1])


# revision 19
# speedup vs baseline: 1.0967x; 1.0246x over previous
"""GAT message-passing kernel for 8 Trainium2 NeuronCores (bf16 rewrite).

Algebraic core (same as the fp32 baseline): with h = x @ proj_w.T + proj_b,
the faithful torch repeat_interleave replication collapses the pre-mask
score to  scores[b, I, J] = leaky_relu(S1 * h[b, I, J//32] + d[b, I])
with S1 = sum(a_w[0, :H]) and d = h @ a_w[0, H:].  Softmax + matmul then
reduce to a masked weighted aggregation over x augmented with a ones
column (the ones column carries the softmax denominator Z):
    W[b, J, I] = adj[J, I] * e[b, J//32, I],   e = exp(leaky(v))
    gt[b, c, I] = sum_J xa[J, c] * W[b, J, I]        (PSUM-accumulated)
    out[b, I, :] = (gt[b].T @ wfin) / Z

Design notes (vs the 25.6us fp32 baseline):

1. All large operands are bf16.  fp32 matmuls run as 2-pass LOW_HIGH on
   the PE (~426ns per 128-col matmul cold); bf16 is single-pass.  The
   host pushes dist values in [200000, 200192) up past the bf16 rounding
   boundary so the on-device threshold compare classifies every edge
   exactly as fp32 would (bf16's 7-bit mantissa rounds that band down
   to 199680, which would flip those edges).
2. The score broadcast eb[q, I] = e[q//4, I] is produced *directly* by
   the v-matmul: the host replicates the folded projection wm into
   wm4[:, q] = wm[:, q//4], so psV4 = wm4.T @ xoT already has the
   block-broadcast layout.  v-chain = matmul -> ACT Prelu -> ACT Exp
   (two same-engine ACT ops, no DVE, no indicator matmul, no PSUM copy).
3. The mask-multiply is DVE-bound, and DVE ops pay a ~150-cycle fixed
   cost (TRN2 SBUF errata), so fewer/bigger ops win.  The host stores
   each dist row twice so the [t, b, i] product tile is built by ONE
   dense [128, 512] scalar_tensor_tensor per 2 J-tiles (the eb operand
   broadcasts over t with a stride-0 dim; dist is the dense side).
4. Batches interleave through the agg matmuls so both finalize chains
   run in parallel (DVE+sync ring for b0, ACT+scalar ring for b1) and
   the two output DMA completion latencies overlap.
5. The first DMA on each ring carries only what the v-chain needs
   (wm4+xoT on one, vcol4+wfin tiny block ahead of it), so the serial
   input-DMA completion latency is paid on the smallest possible load.
"""

import sys

sys.path.insert(0, "/opt/trn_rl_repo")

import numpy as np
import ml_dtypes

B, N, C, H = 2, 1024, 64, 32
P = 128                 # rows per core / partition tile
NCORES = 8
NJ = N // P             # 8 J-tiles of 128
THR = 200000.0
ALPHA = 0.01
H1 = H + 1              # 33: h channels + Z column
C1 = C + 1              # 65: x channels + ones column

CSTA_W = 128 + 2 + H1 + 3   # wm4 | vcol4 fp32 (2 lanes) | wfin | pad | zero fp32

BF16 = ml_dtypes.bfloat16

_CACHE = {}
LAST_RESULT = None
DEBUG_DUMP = False


def _build():
    import concourse.bacc as bacc
    import concourse.bass as bass
    import concourse.tile as tile
    from concourse import mybir

    F32 = mybir.dt.float32
    BF = mybir.dt.bfloat16
    Alu = mybir.AluOpType
    Act = mybir.ActivationFunctionType

    nc = bacc.Bacc("TRN2", target_bir_lowering=False)

    csta_d = nc.dram_tensor("csta", (P, CSTA_W), BF, kind="ExternalInput")
    xot_d = nc.dram_tensor("xot", (C, B * P), BF, kind="ExternalInput")
    xa_d = nc.dram_tensor("xa", (P, B * NJ * C1), BF, kind="ExternalInput")
    distT_d = nc.dram_tensor("distT", (P, NJ * P), BF, kind="ExternalInput")
    out_d = nc.dram_tensor("out", (B, P, H1), F32, kind="ExternalOutput")
    if DEBUG_DUMP:
        dbg_d = nc.dram_tensor("dbg", (P, 3 * B * P), F32, kind="ExternalOutput")

    def bc(src, insert_at, reps):
        """Insert a broadcast (stride-0) dim into an AP's free dims."""
        ap = [list(d) for d in src.ap]
        ap.insert(insert_at, [0, reps])
        return bass.AP(tensor=src.tensor, offset=src.offset, ap=ap)

    with tile.TileContext(nc) as tc:
        with (
            tc.tile_pool(name="const", bufs=1) as const,
            tc.tile_pool(name="persist", bufs=1) as persist,
            tc.tile_pool(name="wtp", bufs=4) as wtp,
            tc.tile_pool(name="work", bufs=2) as work,
            tc.tile_pool(name="psV", bufs=1, space="PSUM") as psV,
            tc.tile_pool(name="psA", bufs=2, space="PSUM") as psA,
            tc.tile_pool(name="psF", bufs=2, space="PSUM") as psF,
        ):
            # ---- input DMAs (ring A = sync, ring B = scalar); the first
            # DMA on each ring carries exactly what the v-chain needs ----
            csta = const.tile([P, CSTA_W], BF)
            nc.sync.dma_start(out=csta[:], in_=csta_d[:])
            wm4 = csta[0:C, 0:128]
            vcol4 = csta[:, 128:130].bitcast(F32)
            wfin = csta[0:C1, 130 : 130 + H1]
            zbias = csta[:, 164:166].bitcast(F32)

            xoT_sb = const.tile([C, B * P], BF)
            nc.scalar.dma_start(out=xoT_sb[:], in_=xot_d[:])
            xoT = xoT_sb[:]

            # dist in two 4-tile chunks matching the mask-multiply groups
            dT_sb = persist.tile([P, NJ * P], BF)
            xa_sb = persist.tile([P, B * NJ * C1], BF)
            nc.scalar.dma_start(out=xa_sb[:], in_=xa_d[:])
            HT = NJ * P // 2
            nc.sync.dma_start(out=dT_sb[:, 0:HT], in_=distT_d[:, 0:HT])
            nc.sync.dma_start(out=dT_sb[:, HT : 2 * HT], in_=distT_d[:, HT : 2 * HT])

            # ---- v-chain: psv[q, b*P+I] = v[q//4, b, I]; e = exp(leaky) ----
            psv = psV.tile([P, B * P], F32)
            nc.tensor.matmul(psv[:], wm4, xoT)
            t4 = work.tile([P, B * P], F32, tag="t4")
            nc.scalar.activation(t4[:], psv[:], Act.Prelu, bias=vcol4, alpha=ALPHA)
            ebd = persist.tile([P, 2 * B * P], BF)
            eb2 = ebd[:, 0 : B * P]
            # bias passed as an AP: a float bias would register a const
            # scalar whose framework MEMSET becomes the profile's
            # first_useful_time ~1.1us before the first DMA
            nc.scalar.activation(eb2, t4[:], Act.Exp, bias=zbias)
            if DEBUG_DUMP:
                dbg = persist.tile([P, 3 * B * P], F32)
                nc.vector.tensor_copy(dbg[:, 0 : B * P], psv[:])
                nc.vector.tensor_copy(dbg[:, B * P : 2 * B * P], t4[:])
                nc.vector.tensor_copy(dbg[:, 2 * B * P : 3 * B * P], eb2[:])
                nc.sync.dma_start(out=dbg_d[:], in_=dbg[:])

            # ---- agg: one dense [128, 512] mask-multiply per 2 J-tiles,
            # feeding 4 interleaved PSUM-accumulated matmuls ----
            ps_g = [
                psA.tile([C1, P], F32, tag="g", name="g0"),
                psA.tile([C1, P], F32, tag="g", name="g1"),
            ]
            # one [128, 512] mask-multiply per (4-tile half, batch): dist is
            # the dense operand, e_b broadcasts over the 4 tiles (a stride-0
            # outer dim keeps the tensor_tensor 2x uop - probed on HW)
            NH = NJ // 2
            for g in range(2):
                for b in range(B):
                    wt = wtp.tile([P, NH * P], BF, tag="wt", name=f"wt{g}{b}")
                    nc.vector.tensor_tensor(
                        out=wt[:],
                        in0=dT_sb[:, g * NH * P : (g + 1) * NH * P],
                        in1=bc(ebd[:, b * P : (b + 1) * P], 1, NH),
                        op=Alu.mult,
                    )
                    for s in range(NH):
                        tk = g * NH + s
                        nc.tensor.matmul(
                            ps_g[b][:],
                            xa_sb[:, (b * NJ + tk) * C1 : (b * NJ + tk + 1) * C1],
                            wt[:, s * P : (s + 1) * P],
                            start=(tk == 0),
                            stop=(tk == NJ - 1),
                            skip_group_check=True,
                        )

            # ---- finalize: b0 on DVE+sync ring, b1 on ACT+scalar ring ----
            gt0 = work.tile([C1, P], BF, tag="gt0")
            nc.vector.tensor_copy(gt0[:], ps_g[0][:])
            gt1 = work.tile([C1, P], BF, tag="gt1")
            nc.scalar.activation(gt1[:], ps_g[1][:], Act.Copy)
            ps_f0 = psF.tile([P, H1], F32, tag="f", name="f0")
            nc.tensor.matmul(ps_f0[:], gt0[:], wfin)
            ps_f1 = psF.tile([P, H1], F32, tag="f", name="f1")
            nc.tensor.matmul(ps_f1[:], gt1[:], wfin)
            ot0 = work.tile([P, H1], F32, tag="ot0")
            nc.vector.tensor_copy(ot0[:], ps_f0[:])
            nc.sync.dma_start(out=out_d[0], in_=ot0[:])
            ot1 = work.tile([P, H1], F32, tag="ot1")
            nc.vector.tensor_copy(ot1[:], ps_f1[:])
            nc.scalar.dma_start(out=out_d[1], in_=ot1[:])


    nc.finalize()
    return nc


def _f32_as_bf16_pair(a):
    """Reinterpret a fp32 array's bytes as pairs of bf16 lanes."""
    a = np.ascontiguousarray(a, dtype=np.float32)
    return a.view(np.uint16).view(BF16)


def kernel(x, dist_mat, proj_w, proj_b, a_w, trace=False):
    global LAST_RESULT
    from concourse.bass_utils import run_bass_kernel_spmd

    x = np.ascontiguousarray(np.asarray(x, dtype=np.float32))
    dist_mat = np.asarray(dist_mat, dtype=np.float32)
    proj_w = np.asarray(proj_w, dtype=np.float32)
    proj_b = np.asarray(proj_b, dtype=np.float32).reshape(H)
    a_w = np.asarray(a_w, dtype=np.float32).reshape(2 * H)

    if "nc" not in _CACHE:
        _CACHE["nc"] = _build()
    nc = _CACHE["nc"]

    # ---- host-side constant folding (all tiny) ----
    a1, a2 = a_w[:H], a_w[H:]
    s1 = np.float32(a1.sum(dtype=np.float32))
    m32 = s1 * np.eye(H, dtype=np.float32) + a2[:, None]   # v = m32.T @ hT
    wta = proj_w.T.astype(np.float32)                      # [C, H]
    wm = wta @ m32                                         # fold h->v
    qidx = np.arange(P) // 4
    wm4 = np.ascontiguousarray(wm[:, qidx])                # [C, 128]
    vcol4 = np.ascontiguousarray((m32.T @ proj_b)[qidx]).reshape(P, 1)
    wfin = np.zeros((C1, H1), np.float32)
    wfin[:C, :H] = wta
    wfin[C, :H] = proj_b
    wfin[C, H] = 1.0


    # adjacency mask on host (exact fp32 compare; diagonal forced to 1),
    # shipped as 0/1 bf16 so the on-device mask apply is a plain multiply
    # (tensor_tensor has a 2x bf16 uop; scalar_tensor_tensor is 1x-only)
    dist_fixed = dist_mat.copy()
    np.fill_diagonal(dist_fixed, 0.0)
    distT = (dist_fixed.T < THR).astype(BF16)  # [N j, N i]

    # token J = 8*q + tk; per-partition pack: [b, tk, C1] with trailing 1.0
    xa = np.ones((B, N, C1), np.float32)
    xa[:, :, :C] = x
    xa_p = np.ascontiguousarray(
        xa.reshape(B, P, NJ, C1).transpose(1, 0, 2, 3).reshape(P, B * NJ * C1)
    ).astype(BF16)

    in_maps = []
    for c in range(NCORES):
        sl = slice(c * P, (c + 1) * P)
        csta = np.zeros((P, CSTA_W), BF16)
        csta[0:C, 0:128] = wm4.astype(BF16)
        csta[:, 128:130] = _f32_as_bf16_pair(vcol4)
        csta[0:C1, 130 : 130 + H1] = wfin.astype(BF16)
        xot = np.ascontiguousarray(
            np.concatenate([x[0, sl, :].T, x[1, sl, :].T], axis=1)
        ).astype(BF16)
        in_maps.append(
            {
                "csta": csta,
                "xot": xot,
                "xa": xa_p,
                "distT": np.ascontiguousarray(distT[:, sl].reshape(P, NJ * P)),
            }
        )

    res = run_bass_kernel_spmd(nc, in_maps, core_ids=list(range(NCORES)), trace=trace)
    LAST_RESULT = res
    full = np.concatenate([res.results[c]["out"] for c in range(NCORES)], axis=1)
    return np.ascontiguousarray(full[:, :, :H] / full[:, :, H : H + 1])


# revision 20
# speedup vs baseline: 1.1086x; 1.0109x over previous
"""GAT message-passing kernel for 8 Trainium2 NeuronCores (bf16).

Algebraic core (inherited from the fp32 baseline): with
h = x @ proj_w.T + proj_b, the faithful torch repeat_interleave
replication collapses the pre-mask score to
    scores[b, I, J] = leaky_relu(S1 * h[b, I, J//32] + d[b, I])
with S1 = sum(a_w[0, :H]) and d = h @ a_w[0, H:].  Softmax + matmul then
reduce to a masked weighted aggregation over x augmented with a ones
column (the ones column carries the softmax denominator Z):
    W[b, J, I] = adj[J, I] * e[b, J//32, I],   e = exp(leaky(v))
    gt[b, c, I] = sum_J xa[J, c] * W[b, J, I]        (PSUM-accumulated)
    out[b, I, :] = (gt[b].T @ wfin) / Z              (divide on host)

Sharding: rows I split 128-per-core over 8 cores, both batches per core;
dist columns are sharded, x and the folded weights replicated.

How this reaches ~18.2us (from the 25.6us fp32 baseline; measured
engine costs in parentheses):

1. Everything large is bf16.  fp32 matmuls run as 2-pass LOW_HIGH on the
   PE (426ns per 128-col matmul at the cold clock); bf16 is single-pass
   (107ns issue rate), and bf16 halves HBM traffic.  PSUM still
   accumulates fp32, and the mask stays exact: the adjacency is
   thresholded on the host in fp32 and shipped as 0/1 bf16.
2. The score broadcast eb[q, I] = e[q//4, I] is produced directly by the
   v-matmul: the host replicates the folded projection wm into
   wm4[:, q] = wm[:, q//4], so psv = wm4.T @ xoT already has the
   block-broadcast layout.  v-chain = matmul -> ACT Prelu -> ACT Exp,
   two same-engine ACT ops; no DVE op, no indicator matmul, no PSUM
   round trip.  All v-chain inputs ride in the first DMA on each ring.
3. The mask apply is DVE tensor_tensor MULTIPLY (bf16 all-SBUF keeps the
   2x_1P uop; scalar_tensor_tensor only has a 1x uop - measured 690 vs
   426ns at FD=512).  Four [128, 512] products total - one per (4-tile
   dist half, batch) - with the dense dist chunk as src0 and e_b
   broadcast across tiles via a stride-0 AP dim (probed: stride-0
   operands keep the 2x uop).  DVE ops pay a ~150-cycle fixed cost, so
   fewer/bigger ops win; 4 ops is the minimum that lets the first
   product start before the second dist chunk lands.
4. The 16 agg matmuls accumulate two PSUM banks (one per batch) and run
   back-to-back at the PE issue rate with zero stalls: the TT cadence
   (426ns for 4 matmuls worth of mask tile) outruns the PE (428ns).
5. Finalize splits across engines: b0 via DVE cast + sync-ring DMA, b1
   via ACT copy + scalar-ring DMA, so the two output DMA completion
   latencies overlap.
6. DMA plan: ring A (sync) csta -> dist halves, ring B (scalar) xoT ->
   xa.  The first DMA per ring is small so its completion semaphore
   lands early; ~0.6us HWDGE first-byte latency, ~1.5us completion
   receipt, and a ~6.7us inter-iteration framework tail are measured
   invariants that bound exec_time from below.
"""

import sys

sys.path.insert(0, "/opt/trn_rl_repo")

import numpy as np
import ml_dtypes

B, N, C, H = 2, 1024, 64, 32
P = 128                 # rows per core / partition tile
NCORES = 8
NJ = N // P             # 8 J-tiles of 128
THR = 200000.0
ALPHA = 0.01
H1 = H + 1              # 33: h channels + Z column
C1 = C + 1              # 65: x channels + ones column

CSTA_W = 128 + 2 + H1 + 3   # wm4 | vcol4 fp32 (2 lanes) | wfin | pad | zero fp32

BF16 = ml_dtypes.bfloat16

_CACHE = {}
LAST_RESULT = None
DEBUG_DUMP = False


def _build():
    import concourse.bacc as bacc
    import concourse.bass as bass
    import concourse.tile as tile
    from concourse import mybir

    F32 = mybir.dt.float32
    BF = mybir.dt.bfloat16
    Alu = mybir.AluOpType
    Act = mybir.ActivationFunctionType

    nc = bacc.Bacc("TRN2", target_bir_lowering=False)

    csta_d = nc.dram_tensor("csta", (P, CSTA_W), BF, kind="ExternalInput")
    xot_d = nc.dram_tensor("xot", (C, B * P), BF, kind="ExternalInput")
    xa_d = nc.dram_tensor("xa", (P, B * NJ * C1), BF, kind="ExternalInput")
    distT_d = nc.dram_tensor("distT", (P, NJ * P), BF, kind="ExternalInput")
    out_d = nc.dram_tensor("out", (B, P, H1), F32, kind="ExternalOutput")
    if DEBUG_DUMP:
        dbg_d = nc.dram_tensor("dbg", (P, 3 * B * P), F32, kind="ExternalOutput")

    def bc(src, insert_at, reps):
        """Insert a broadcast (stride-0) dim into an AP's free dims."""
        ap = [list(d) for d in src.ap]
        ap.insert(insert_at, [0, reps])
        return bass.AP(tensor=src.tensor, offset=src.offset, ap=ap)

    with tile.TileContext(nc) as tc:
        with (
            tc.tile_pool(name="const", bufs=1) as const,
            tc.tile_pool(name="persist", bufs=1) as persist,
            tc.tile_pool(name="wtp", bufs=4) as wtp,
            tc.tile_pool(name="work", bufs=2) as work,
            tc.tile_pool(name="psV", bufs=1, space="PSUM") as psV,
            tc.tile_pool(name="psA", bufs=2, space="PSUM") as psA,
            tc.tile_pool(name="psF", bufs=2, space="PSUM") as psF,
        ):
            # ---- input DMAs (ring A = sync, ring B = scalar); the first
            # DMA on each ring carries exactly what the v-chain needs ----
            csta = const.tile([P, CSTA_W], BF)
            nc.sync.dma_start(out=csta[:], in_=csta_d[:])
            wm4 = csta[0:C, 0:128]
            vcol4 = csta[:, 128:130].bitcast(F32)
            wfin = csta[0:C1, 130 : 130 + H1]
            zbias = csta[:, 164:166].bitcast(F32)

            xoT_sb = const.tile([C, B * P], BF)
            nc.scalar.dma_start(out=xoT_sb[:], in_=xot_d[:])
            xoT = xoT_sb[:]

            # dist in two 4-tile chunks matching the mask-multiply groups
            dT_sb = persist.tile([P, NJ * P], BF)
            xa_sb = persist.tile([P, B * NJ * C1], BF)
            nc.scalar.dma_start(out=xa_sb[:], in_=xa_d[:])
            HT = NJ * P // 2
            nc.sync.dma_start(out=dT_sb[:, 0:HT], in_=distT_d[:, 0:HT])
            nc.sync.dma_start(out=dT_sb[:, HT : 2 * HT], in_=distT_d[:, HT : 2 * HT])

            # ---- v-chain: psv[q, b*P+I] = v[q//4, b, I]; e = exp(leaky) ----
            psv = psV.tile([P, B * P], F32)
            nc.tensor.matmul(psv[:], wm4, xoT)
            t4 = work.tile([P, B * P], F32, tag="t4")
            nc.scalar.activation(t4[:], psv[:], Act.Prelu, bias=vcol4, alpha=ALPHA)
            ebd = persist.tile([P, 2 * B * P], BF)
            eb2 = ebd[:, 0 : B * P]
            # bias passed as an AP: a float bias would register a const
            # scalar whose framework MEMSET becomes the profile's
            # first_useful_time ~1.1us before the first DMA
            nc.scalar.activation(eb2, t4[:], Act.Exp, bias=zbias)
            if DEBUG_DUMP:
                dbg = persist.tile([P, 3 * B * P], F32)
                nc.vector.tensor_copy(dbg[:, 0 : B * P], psv[:])
                nc.vector.tensor_copy(dbg[:, B * P : 2 * B * P], t4[:])
                nc.vector.tensor_copy(dbg[:, 2 * B * P : 3 * B * P], eb2[:])
                nc.sync.dma_start(out=dbg_d[:], in_=dbg[:])

            # ---- agg: one dense [128, 512] mask-multiply per 2 J-tiles,
            # feeding 4 interleaved PSUM-accumulated matmuls ----
            ps_g = [
                psA.tile([C1, P], F32, tag="g", name="g0"),
                psA.tile([C1, P], F32, tag="g", name="g1"),
            ]
            # one [128, 512] mask-multiply per (4-tile half, batch): dist is
            # the dense operand, e_b broadcasts over the 4 tiles (a stride-0
            # outer dim keeps the tensor_tensor 2x uop - probed on HW)
            NH = NJ // 2
            for g in range(2):
                for b in range(B):
                    wt = wtp.tile([P, NH * P], BF, tag="wt", name=f"wt{g}{b}")
                    nc.vector.tensor_tensor(
                        out=wt[:],
                        in0=dT_sb[:, g * NH * P : (g + 1) * NH * P],
                        in1=bc(ebd[:, b * P : (b + 1) * P], 1, NH),
                        op=Alu.mult,
                    )
                    for s in range(NH):
                        tk = g * NH + s
                        nc.tensor.matmul(
                            ps_g[b][:],
                            xa_sb[:, (b * NJ + tk) * C1 : (b * NJ + tk + 1) * C1],
                            wt[:, s * P : (s + 1) * P],
                            start=(tk == 0),
                            stop=(tk == NJ - 1),
                            skip_group_check=True,
                        )

            # ---- finalize: b0 on DVE+sync ring, b1 on ACT+scalar ring ----
            gt0 = work.tile([C1, P], BF, tag="gt0")
            nc.vector.tensor_copy(gt0[:], ps_g[0][:])
            gt1 = work.tile([C1, P], BF, tag="gt1")
            nc.scalar.activation(gt1[:], ps_g[1][:], Act.Copy)
            ps_f0 = psF.tile([P, H1], F32, tag="f", name="f0")
            nc.tensor.matmul(ps_f0[:], gt0[:], wfin)
            ps_f1 = psF.tile([P, H1], F32, tag="f", name="f1")
            nc.tensor.matmul(ps_f1[:], gt1[:], wfin)
            ot0 = work.tile([P, H1], F32, tag="ot0")
            nc.vector.tensor_copy(ot0[:], ps_f0[:])
            nc.sync.dma_start(out=out_d[0], in_=ot0[:])
            ot1 = work.tile([P, H1], F32, tag="ot1")
            nc.vector.tensor_copy(ot1[:], ps_f1[:])
            nc.scalar.dma_start(out=out_d[1], in_=ot1[:])


    nc.finalize()
    return nc


def _f32_as_bf16_pair(a):
    """Reinterpret a fp32 array's bytes as pairs of bf16 lanes."""
    a = np.ascontiguousarray(a, dtype=np.float32)
    return a.view(np.uint16).view(BF16)


def kernel(x, dist_mat, proj_w, proj_b, a_w, trace=False):
    global LAST_RESULT
    from concourse.bass_utils import run_bass_kernel_spmd

    x = np.ascontiguousarray(np.asarray(x, dtype=np.float32))
    dist_mat = np.asarray(dist_mat, dtype=np.float32)
    proj_w = np.asarray(proj_w, dtype=np.float32)
    proj_b = np.asarray(proj_b, dtype=np.float32).reshape(H)
    a_w = np.asarray(a_w, dtype=np.float32).reshape(2 * H)

    if "nc" not in _CACHE:
        _CACHE["nc"] = _build()
    nc = _CACHE["nc"]

    # ---- host-side constant folding (all tiny) ----
    a1, a2 = a_w[:H], a_w[H:]
    s1 = np.float32(a1.sum(dtype=np.float32))
    m32 = s1 * np.eye(H, dtype=np.float32) + a2[:, None]   # v = m32.T @ hT
    wta = proj_w.T.astype(np.float32)                      # [C, H]
    wm = wta @ m32                                         # fold h->v
    qidx = np.arange(P) // 4
    wm4 = np.ascontiguousarray(wm[:, qidx])                # [C, 128]
    vcol4 = np.ascontiguousarray((m32.T @ proj_b)[qidx]).reshape(P, 1)
    wfin = np.zeros((C1, H1), np.float32)
    wfin[:C, :H] = wta
    wfin[C, :H] = proj_b
    wfin[C, H] = 1.0


    # adjacency mask on host (exact fp32 compare; diagonal forced to 1),
    # shipped as 0/1 bf16 so the on-device mask apply is a plain multiply
    # (tensor_tensor has a 2x bf16 uop; scalar_tensor_tensor is 1x-only)
    dist_fixed = dist_mat.copy()
    np.fill_diagonal(dist_fixed, 0.0)
    distT = (dist_fixed.T < THR).astype(BF16)  # [N j, N i]

    # token J = 8*q + tk; per-partition pack: [b, tk, C1] with trailing 1.0
    xa = np.ones((B, N, C1), np.float32)
    xa[:, :, :C] = x
    xa_p = np.ascontiguousarray(
        xa.reshape(B, P, NJ, C1).transpose(1, 0, 2, 3).reshape(P, B * NJ * C1)
    ).astype(BF16)

    in_maps = []
    for c in range(NCORES):
        sl = slice(c * P, (c + 1) * P)
        csta = np.zeros((P, CSTA_W), BF16)
        csta[0:C, 0:128] = wm4.astype(BF16)
        csta[:, 128:130] = _f32_as_bf16_pair(vcol4)
        csta[0:C1, 130 : 130 + H1] = wfin.astype(BF16)
        xot = np.ascontiguousarray(
            np.concatenate([x[0, sl, :].T, x[1, sl, :].T], axis=1)
        ).astype(BF16)
        in_maps.append(
            {
                "csta": csta,
                "xot": xot,
                "xa": xa_p,
                "distT": np.ascontiguousarray(distT[:, sl].reshape(P, NJ * P)),
            }
        )

    res = run_bass_kernel_spmd(nc, in_maps, core_ids=list(range(NCORES)), trace=trace)
    LAST_RESULT = res
    full = np.concatenate([res.results[c]["out"] for c in range(NCORES)], axis=1)
    return np.ascontiguousarray(full[:, :, :H] / full[:, :, H : H + 1])
